# revision 28
# baseline (speedup 1.0000x reference)
"""Trainium2 Bass kernel for nn_DiffusionModel1d (batched 1-D diffusion solve).

Math: the reference solves A(K) u = f per batch row with K = exp(x) via the
Thomas algorithm, where A = G^T diag(K_hat) G, G the n x n lower-bidiagonal
difference matrix (1 on diag, -1 on subdiag) and
K_hat = (2*K_0, K_1, ..., K_{n-1}).  Hence

    u = h2 * G^{-1} diag(K_hat)^{-1} G^{-T} f
      = h2 * cumsum_j( w_j * exp(-x_j) ),   w = suffix_sum(f), w_0 halved.

So the whole solve is: one exp, one elementwise multiply by a shared
per-column vector, and one hardware prefix-sum scan along the grid dim.
Pure data parallel over batch: 8192 rows -> 1024 rows per core x 8 cores.

Engine split per 128-row group (8 groups per core):
  SP    : most x loads (HWDGE queue 1), 2047 of 2048 cols (last col unused)
  ACT   : exp(-x) -> bf16, last group's load halves mid-stream, and the
          late half/quarter stores (HWDGE queue 2)
  DVE   : vt = et * wb (bf16 2x), paired scan -> odd prefix outputs (fp32)
  Pool  : evens fill u[2k] = u[2k-1] + v[2k] (covers u[0] via a permanent
          zero column at ut[:, 0]) + early stores on its own SWDGE queue,
          so no store's sem wait can stall the ACT exp chain or DVE
  PE    : one-time broadcast of the 4KB w [16,128] tile to [128, 2047] via
          row-selector matmuls (replaces a 524KB replicated HBM load; a
          [1, N] DMA would emit per-element descriptors)

HW-probed DMA behavior (dma_probe.py + bench_hw.py repeat-NEFF slopes):
reads sustain ~428 GB/s/core, writes ~211 GB/s, and any read+write mix
lands at ~60 us for the mandatory 16.8 MB regardless of queue/phase
structure -- a shared weighted budget where writes cost ~2x reads.  The
kernel is therefore at the DMA floor; this schedule wins vs the single
queue baseline via 524KB fewer read bytes, an earlier first store, and a
ramp/tail that never head-of-line-blocks a queue (cost model: 36.2us vs
53.0us; the measurable one-shot HW gain is the ramp/tail + wb slice).
"""

import os
import sys

import numpy as np

sys.path.insert(0, "/opt/trn_rl_repo")

import ml_dtypes

import concourse.bacc as bacc
import concourse.mybir as mybir
import concourse.tile as tile
from concourse import bass_utils

B, M = 8192, 2048
N = M - 1
NCORES = 8
BC = B // NCORES          # 1024 batch rows per core
P = 128                   # SBUF partitions
GROUPS = BC // P          # 8 partition-groups per core
H2 = (1.0 / N) ** 2

_cached_nc = None
LAST_RESULTS = None

# ---- schedule configuration (tuned against the CoreSim cost model) ----
# loads: group -> list of (engine, emit_at, c0, c1) column chunks; "sp"
# chunks are hoisted before the group loop, "act" chunks are emitted on the
# ACT queue inside group emit_at's loop body (between exps).
HALF = 1024
LOAD_ENG = {g: [("sp", None, 0, N)] for g in range(GROUPS)}
LOAD_ENG[0] = [("sp", None, 0, HALF), ("sp", None, HALF, N)]
LOAD_ENG[GROUPS - 1] = [("act", 2, 0, HALF), ("act", 4, HALF, N)]
# exp/mult column chunks per group
EM_SPLITS = {0: 2, GROUPS - 1: 2}
# scan/evens/store column chunks per group (1 = whole row, 2 = halves)
SPLITS = {0: 2, GROUPS - 2: 2, GROUPS - 1: 4}
# store engine per (group, chunk-index); fallback key: group
STORE_ENG = {0: "pool", 1: "pool", 2: "pool", 3: "pool",
             4: "sp", 5: "sp",
             (6, 0): "act", (6, 1): "pool",
             (7, 0): "act", (7, 1): "pool",
             (7, 2): "act", (7, 3): "pool"}
# benchmarking only: unroll the whole pipeline this many times in one NEFF
REPEAT = 1
# benchmarking only: all-engine barrier between reps (one-shot approximation)
BARRIER_REPS = False


def _build_kernel():
    fp32 = mybir.dt.float32
    bf16 = mybir.dt.bfloat16
    nc = bacc.Bacc(
        "TRN2",
        target_bir_lowering=False,
        debug=False,
        enable_asserts=False,
        num_devices=NCORES,
    )
    x_d = nc.dram_tensor("x", (BC, M), fp32, kind="ExternalInput").ap()
    w_d = nc.dram_tensor("w", (16, P), bf16, kind="ExternalInput").ap()
    o_d = nc.dram_tensor("out", (BC, N), fp32, kind="ExternalOutput").ap()

    add = mybir.AluOpType.add

    with tile.TileContext(nc) as tc:
        with (
            tc.tile_pool(name="const", bufs=1) as cpool,
            tc.tile_pool(name="xin", bufs=GROUPS) as xpool,
            tc.tile_pool(name="ew", bufs=3) as epool,
            tc.psum_pool(name="ps", bufs=1) as ppool,
        ):
            # ---- one-time w broadcast: 4KB [16, 128] load -> [P, N] bf16
            # via 16 row-selector matmuls (lhsT = identity column k,
            # broadcast along the free dim; rhs = w16).  A [1, N] DMA would
            # emit per-element descriptors; [16, 128] is clean.
            w16 = cpool.tile([16, P], bf16, tag="w16")
            nc.scalar.dma_start(out=w16, in_=w_d)
            # identity16 = is_equal(c - p, 0) from an iota ramp
            ramp = cpool.tile([16, 16], mybir.dt.int32, tag="ramp")
            nc.gpsimd.iota(ramp, [[1, 16]], channel_multiplier=-1)
            i16 = cpool.tile([16, 16], bf16, tag="i16")
            nc.vector.tensor_scalar(
                out=i16,
                in0=ramp,
                scalar1=0,
                scalar2=None,
                op0=mybir.AluOpType.is_equal,
            )
            pw = ppool.tile([P, N], fp32, tag="pw")
            for k in range(16):
                c0 = k * P
                cols = min(P, N - c0)
                nc.tensor.matmul(
                    pw[:, c0 : c0 + cols],
                    i16[:, k : k + 1].broadcast_to([16, P]),
                    w16[:, :cols],
                    start=True,
                    stop=True,
                )
            # copy PSUM -> SBUF in halves so group 0's mult can start as
            # soon as the first 8 blocks are broadcast
            wb = cpool.tile([P, N], bf16, tag="wb")
            nc.vector.tensor_copy(out=wb[:, : 8 * P], in_=pw[:, : 8 * P])
            nc.vector.tensor_copy(out=wb[:, 8 * P :], in_=pw[:, 8 * P :])

            # ---- persistent prefix tiles: col 0 is a permanent zero so the
            # evens fill u[2k] = u[2k-1] + v[2k] also covers u[0] = 0 + v[0].
            NUBUF = 4
            uts = []
            for b in range(NUBUF):
                ut = cpool.tile([P, M], fp32, tag=f"u{b}")
                nc.gpsimd.memset(ut[:, 0:1], 0.0)
                uts.append(ut)

            # ---- input loads: most hoisted on the SP queue (in-order ring
            # with no waits); some ride ACT mid-stream (emitted in a later
            # loop body) to shorten the serial SP load chain.  Group 0 is
            # split so its exp starts half early.
            engs = {"pool": nc.gpsimd, "sp": nc.sync, "act": nc.scalar}

            def chunks(g, ns):
                bounds = [0] + [
                    (((N * (i + 1)) // ns) // 2) * 2 for i in range(ns - 1)
                ] + [N]
                return list(zip(bounds[:-1], bounds[1:]))

            for _rep in range(REPEAT):
                if BARRIER_REPS and _rep:
                    nc.all_engine_barrier()
                _emit_pipeline(
                    nc, tc, engs, xpool, epool, uts, wb, x_d, o_d, chunks
                )

    nc.compile()
    return nc


def _emit_pipeline(nc, tc, engs, xpool, epool, uts, wb, x_d, o_d, chunks):
    fp32 = mybir.dt.float32
    bf16 = mybir.dt.bfloat16
    add = mybir.AluOpType.add
    NUBUF = len(uts)
    xts = [
        xpool.tile([P, N], fp32, tag="x", name="xt") for _ in range(GROUPS)
    ]
    for g in range(GROUPS):
        for eng, at, c0, c1 in LOAD_ENG[g]:
            if at is None:
                engs[eng].dma_start(
                    out=xts[g][:, c0:c1],
                    in_=x_d[g * P : (g + 1) * P, c0:c1],
                )

    # ---- per-group pipeline: exp -> mult -> paired scan (odds) ->
    # Pool evens fill -> store.  u[j] lives at ut[:, j+1].
    for g in range(GROUPS):
        for lg in range(GROUPS):
            for eng, at, c0, c1 in LOAD_ENG[lg]:
                if at == g:
                    engs[eng].dma_start(
                        out=xts[lg][:, c0:c1],
                        in_=x_d[lg * P : (lg + 1) * P, c0:c1],
                    )
        rows = slice(g * P, (g + 1) * P)
        xt = xts[g]
        ut = uts[g % NUBUF]
        et = epool.tile([P, N], bf16, tag="e")
        vt = epool.tile([P, N], bf16, tag="v")
        for c0, c1 in chunks(g, EM_SPLITS.get(g, 1)):
            nc.scalar.activation(
                out=et[:, c0:c1],
                in_=xt[:, c0:c1],
                func=mybir.ActivationFunctionType.Exp,
                scale=-1.0,
            )
            nc.vector.tensor_mul(
                out=vt[:, c0:c1], in0=et[:, c0:c1], in1=wb[:, c0:c1]
            )
        for si, (c0, c1) in enumerate(chunks(g, SPLITS.get(g, 1))):
            # paired scan: state absorbs TWO elements per step
            # (state = (v[2t] + state) + v[2t+1]), writing prefix
            # values at odd grid positions: ut[:, 2t+2] = P[2t+1].
            npairs = (c1 - c0) // 2
            nc.vector.tensor_tensor_scan(
                out=ut[:, c0 + 2 : c0 + 2 * npairs + 1 : 2],
                data0=vt[:, c0 : c0 + 2 * npairs : 2],
                data1=vt[:, c0 + 1 : c0 + 2 * npairs : 2],
                initial=0.0 if si == 0 else ut[:, c0 : c0 + 1],
                op0=add,
                op1=add,
            )
            # evens fill on Pool: u[2k] = u[2k-1] + v[2k]; col 0 of
            # ut is the permanent zero, so k=0 gives u[0] = v[0].
            nev = (c1 - c0 + 1) // 2
            nc.gpsimd.tensor_tensor(
                out=ut[:, c0 + 1 : c0 + 2 * nev : 2],
                in0=ut[:, c0 : c0 + 2 * nev - 1 : 2],
                in1=vt[:, c0 : c0 + 2 * nev - 1 : 2],
                op=add,
            )
            # Spread the serial store stream across all three
            # DMA-capable engines so none becomes the bottleneck
            # (early on Pool — same engine as the evens fill, no
            # cross-engine wait; later ones on SP after its loads
            # drain and on ACT after its exps are done).
            eng = engs[STORE_ENG.get((g, si), STORE_ENG.get(g))]
            eng.dma_start(
                out=o_d[rows, c0:c1], in_=ut[:, c0 + 1 : c1 + 1]
            )


def _get_nc():
    global _cached_nc
    if _cached_nc is None:
        _cached_nc = _build_kernel()
    return _cached_nc


def _make_w(f_rhs: np.ndarray) -> np.ndarray:
    """w = h2 * suffix_sum(f), w[0] halved; bf16, padded to [16, 128]."""
    w = np.cumsum(f_rhs[::-1].astype(np.float64))[::-1] * H2
    w[0] *= 0.5
    wp = np.zeros(16 * P, dtype=np.float64)
    wp[:N] = w
    return np.ascontiguousarray(wp.astype(ml_dtypes.bfloat16).reshape(16, P))


def kernel(x: np.ndarray, f_rhs: np.ndarray) -> np.ndarray:
    assert x.shape == (B, M) and f_rhs.shape == (N,)
    x = np.ascontiguousarray(x, dtype=np.float32)
    wb = _make_w(np.asarray(f_rhs, dtype=np.float32))
    nc = _get_nc()
    in_maps = [
        {"x": x[c * BC : (c + 1) * BC], "w": wb} for c in range(NCORES)
    ]
    res = bass_utils.run_bass_kernel_spmd(
        nc,
        in_maps,
        core_ids=list(range(NCORES)),
        trace=bool(int(os.environ.get("KERNEL_TRACE", "0"))),
    )
    global LAST_RESULTS
    LAST_RESULTS = res
    out = np.concatenate(
        [res.results[c]["out"] for c in range(NCORES)], axis=0
    ).astype(np.float32)
    return out
